# revision 37
# baseline (speedup 1.0000x reference)
"""K2Layer Trainium2 kernel: RMSNorm -> gated causal conv + low-rank decayed
linear attention -> proj -> residual -> RMSNorm -> MLP -> residual.

Sharding: pure data-parallel over batch (B=8 -> 1 batch element per core),
all parameters replicated.

v2 design notes (vs the SBUF-accumulation baseline):
- MLP second GEMM accumulates over the 32 f-tiles in PSUM (start/stop
  accumulation groups) instead of 512 DVE adds into SBUF; MLP weights,
  activations and the transposed inputs are bf16 (half DMA, 1024-wide
  moving operands, 1 cycle/row everywhere).
- proj is folded into the attention values: hv = h^T-tiles @ proj_eff
  reuses the phase-B PE transposes, so the whole mixer runs post-proj and
  the mixer-output transposes + separate proj phase disappear.
- The chunk state recurrence is decoupled from the PE chain: all U_c are
  computed per chunk, a tiny Pool-engine in-place scan produces the
  states, then the 8 chunk mixers are independent.
- rmsnorm row scales s_j are folded into the j-side chunk matrices
  (M, kbT, Bp) so the normalized activations are never materialized; the
  residual accumulates in place into the h tiles (h2 == hpb).
- Every matmul uses same-dtype operands (bf16xbf16 or f32r x f32r).

Self-contained: shapes hardcoded for B=8, S=1024, D=1024, R=16, K=4.
"""
import copy
import numpy as np
import ml_dtypes

import concourse.bass as bass
import concourse.mybir as mybir
import concourse.tile as tile
from concourse.bass_utils import run_bass_kernel_spmd
from concourse.masks import make_identity

f32 = mybir.dt.float32
fr = mybir.dt.float32r
bf16 = mybir.dt.bfloat16
AF = mybir.ActivationFunctionType
ALU = mybir.AluOpType

B, S, D, R, KK = 8, 1024, 1024, 16, 4
R2 = 32 + R        # padded q|k row count: q at 0:R, k at 32:32+R
F = 4 * D
C = 128            # attention chunk length == tile height
NT = S // 128      # t tiles
ND = D // 128      # d tiles
NF = F // 128      # f tiles
NCH = S // C       # chunks
EPS_RMS = 1e-6
EPS_L2 = 1e-8
GAMMA_MIN, GAMMA_MAX = 0.15, 1.0
ALPHA_CAP = 1.0

_cache = {}


def _sigmoid(x):
    return 1.0 / (1.0 + np.exp(-x))


def _host_prep(inputs):
    u = np.asarray(inputs['u'], np.float64)
    v = np.asarray(inputs['v'], np.float64)
    norm1_w = np.asarray(inputs['norm1_w'], np.float64)
    norm2_w = np.asarray(inputs['norm2_w'], np.float64)
    proj_w = np.asarray(inputs['proj_w'], np.float64)
    mlp_w1 = np.asarray(inputs['mlp_w1'], np.float64)
    ker = np.asarray(inputs['k_base_kernel'], np.float64)

    gate = _sigmoid(float(inputs['k_base_gate_logit']))
    alpha = ALPHA_CAP * _sigmoid(np.asarray(inputs['alpha_logit'], np.float64))
    gamma = np.clip(_sigmoid(np.asarray(inputs['decay_logit'], np.float64)),
                    GAMMA_MIN, GAMMA_MAX)
    assert gamma.min() >= 0.25, "chunked gamma tables overflow fp32 below 0.25"

    p = np.arange(C)
    # 2^-16 on each of Gq/Gk keeps kh*qh products well under f32/bf16 max
    # (gamma^(p-j) reaches gamma^-127); the tril mask constant restores 2^32.
    # Gbar carries 2^16 so the state path (qh * stv) comes out unscaled.
    Gq = alpha[:, None] * gamma[:, None] ** (p[None, :] - 64) * 2.0 ** -16
    Gk = gamma[:, None] ** (64 - p[None, :]) * 2.0 ** -16
    Gbar = gamma[:, None] ** (C + 64 - p[None, :]) * 2.0 ** 16
    Gc = gamma[:, None] ** C

    Bc = np.zeros((C, C))
    for m in range(KK):
        idx = np.arange(C - m)
        Bc[idx, idx + m] = gate * ker[m]
    Bp = np.zeros((3, C))
    for q in range(3):
        for pp in range(3):
            m = pp - q + 3
            if 1 <= m <= 3:
                Bp[q, pp] = gate * ker[m]

    # q occupies partitions 0:R, k partitions 32:32+R (engine partition
    # accesses must start at 32-aligned bases; 16 is illegal)
    eones = np.zeros((R2, 2), np.float64)
    eones[:R, 0] = 1.0
    eones[32:32 + R, 1] = 1.0

    c = lambda x: np.ascontiguousarray(np.asarray(x, np.float32))
    cb = lambda x: np.ascontiguousarray(
        np.asarray(x, np.float32).astype(ml_dtypes.bfloat16))
    w1_eff = mlp_w1 * norm2_w[:, None]                       # [D,F]
    # pack w1 so each [128,128] (dtile,ftile) tile is contiguous for DMA
    w1p = np.ascontiguousarray(
        w1_eff.reshape(ND, 128, NF, 128).transpose(2, 1, 0, 3)
        .reshape(NF, 128, ND * 128).astype(np.float32)).astype(ml_dtypes.bfloat16)
    return dict(
        uv_eff=cb(np.concatenate(
            [u, np.zeros((D, 32 - R)), v], 1) * norm1_w[:, None]),
        proj_eff=cb(proj_w.T * norm1_w[:, None]),
        w1p=w1p,
        w2=cb(inputs['mlp_w2']),
        proj_b=c(np.reshape(inputs['proj_b'], (1, D))),
        b1p=c(np.reshape(inputs['mlp_b1'], (NF, 128)).T),
        b2=cb(np.reshape(inputs['mlp_b2'], (1, D))),
        Gq=c(Gq), Gk=c(Gk), Gbar=c(Gbar), Gc=c(Gc),
        Bc=c(Bc), Bp=c(Bp), eones=c(eones), e2=c(eones.T),
    )


def split_drain_waits(nc):
    """This walrus build allows at most ONE sem wait per instruction (any
    opcode). Peel excess waits onto preceding same-engine NoOp carriers."""
    n = 0
    for f in nc.m.functions:
        for bb in f.blocks:
            i = 0
            while i < len(bb.instructions):
                ins = bb.instructions[i]
                si = ins.sync_info
                if si and si.on_wait and len(si.on_wait) > 1:
                    waits = list(si.on_wait)
                    carriers = []
                    for k, w in enumerate(waits[:-1]):
                        nop = mybir.InstNoOp(name=f"{ins.name}-wpeel{k}", ins=[], outs=[])
                        nop.engine = ins.engine
                        si2 = copy.deepcopy(si)
                        si2.on_wait[:] = [w]
                        si2.on_update[:] = []
                        nop.sync_info = si2
                        carriers.append(nop)
                        n += 1
                    si.on_wait[:] = [waits[-1]]
                    bb.instructions[i:i] = carriers
                    i += len(carriers)
                i += 1
    return n


def _build_nc(skip_pb=False, skip_b2=False, debug=False):
    nc = bass.Bass("TRN2")
    h_d = nc.dram_tensor("h", [S, D], fr, kind="ExternalInput")
    uv_d = nc.dram_tensor("uv_eff", [D, R2], bf16, kind="ExternalInput")
    proj_d = nc.dram_tensor("proj_eff", [D, D], bf16, kind="ExternalInput")
    w1_d = nc.dram_tensor("w1p", [NF, 128, ND * 128], bf16, kind="ExternalInput")
    w2_d = nc.dram_tensor("w2", [F, D], bf16, kind="ExternalInput")
    pb_d = nc.dram_tensor("proj_b", [1, D], fr, kind="ExternalInput")
    b1_d = nc.dram_tensor("b1p", [128, NF], f32, kind="ExternalInput")
    b2_d = nc.dram_tensor("b2", [1, D], bf16, kind="ExternalInput")
    gq_d = nc.dram_tensor("Gq", [R, C], f32, kind="ExternalInput")
    gk_d = nc.dram_tensor("Gk", [R, C], f32, kind="ExternalInput")
    gbar_d = nc.dram_tensor("Gbar", [R, C], f32, kind="ExternalInput")
    gc_d = nc.dram_tensor("Gc", [R, 1], f32, kind="ExternalInput")
    bc_d = nc.dram_tensor("Bc", [C, C], f32, kind="ExternalInput")
    bp_d = nc.dram_tensor("Bp", [3, C], f32, kind="ExternalInput")
    eo_d = nc.dram_tensor("eones", [R2, 2], fr, kind="ExternalInput")
    e2_d = nc.dram_tensor("e2", [2, R2], fr, kind="ExternalInput")
    y_d = nc.dram_tensor("y", [S, D], fr, kind="ExternalOutput")
    if debug:
        dbg_qnT = nc.dram_tensor("dbg_qnT", [R2, S], bf16, kind="ExternalOutput")
        dbg_qk2 = nc.dram_tensor("dbg_qk2", [R2, S], bf16, kind="ExternalOutput")
        dbg_ssq2 = nc.dram_tensor("dbg_ssq2", [2, S], f32, kind="ExternalOutput")
        dbg_srow = nc.dram_tensor("dbg_srow", [2, S], fr, kind="ExternalOutput")
        dbg_M = nc.dram_tensor("dbg_M", [NCH, C, C], bf16, kind="ExternalOutput")
        dbg_hv = nc.dram_tensor("dbg_hv", [NCH, 128, D], bf16, kind="ExternalOutput")
        dbg_h2 = nc.dram_tensor("dbg_h2", [S, D], fr, kind="ExternalOutput")

    h_r = h_d.rearrange("(n p) d -> n p d", p=128)
    y_r = y_d.rearrange("(n p) d -> n p d", p=128)
    prj_r = proj_d.rearrange("(n p) d -> n p d", p=128)
    w2_r = w2_d.rearrange("(n p) d -> n p d", p=128)
    uv_r = uv_d.rearrange("(n p) r -> n p r", p=128)

    with tile.TileContext(nc) as tc:
        with (
            tc.tile_pool(name="const", bufs=1) as const,
            tc.tile_pool(name="scal", bufs=2) as scal,
            tc.tile_pool(name="scr1", bufs=1) as scr1,
            tc.tile_pool(name="hpb", bufs=1) as hpbp,
            tc.tile_pool(name="srp", bufs=1) as srp,
        ):
            def warm():
                pass
            # ---- constants ----
            ident = const.tile([128, 128], f32, tag="ident", name="ident")
            make_identity(nc, ident)
            ident_r = const.tile([128, 128], fr, tag="identr", name="identr")
            nc.vector.tensor_copy(ident_r, ident)
            ident_b = const.tile([128, 128], bf16, tag="identb", name="identb")
            nc.vector.tensor_copy(ident_b, ident)
            trilm = const.tile([C, C], f32, tag="trilm", name="trilm")
            nc.gpsimd.memset(trilm, 0.0)
            nc.gpsimd.affine_select(
                out=trilm, in_=trilm, compare_op=ALU.is_gt, fill=2.0 ** 32,
                base=0, pattern=[[-1, C]], channel_multiplier=1)
            gq_s = const.tile([R, C], f32, tag="gq", name="gq"); nc.sync.dma_start(out=gq_s, in_=gq_d[:, :])
            gk_s = const.tile([R, C], f32, tag="gk", name="gk"); nc.sync.dma_start(out=gk_s, in_=gk_d[:, :])
            gbar_s = const.tile([R, C], f32, tag="gbar", name="gbar"); nc.sync.dma_start(out=gbar_s, in_=gbar_d[:, :])
            gc_s = const.tile([R, 1], f32, tag="gc", name="gc"); nc.sync.dma_start(out=gc_s, in_=gc_d[:, :])
            bc_s = const.tile([C, C], f32, tag="bc", name="bc"); nc.sync.dma_start(out=bc_s, in_=bc_d[:, :])
            bp_s = const.tile([3, C], f32, tag="bp", name="bp"); nc.sync.dma_start(out=bp_s, in_=bp_d[:, :])
            eo_s = const.tile([R2, 2], fr, tag="eo", name="eo"); nc.sync.dma_start(out=eo_s, in_=eo_d[:, :])
            e2_s = const.tile([2, R2], fr, tag="e2", name="e2"); nc.sync.dma_start(out=e2_s, in_=e2_d[:, :])
            uv_s = const.tile([128, ND, R2], bf16, tag="uv", name="uv")
            for k in range(ND):
                nc.sync.dma_start(out=uv_s[:, k, :], in_=uv_r[k])
            eps1 = const.tile([128, 1], f32, tag="eps1", name="eps1")
            nc.vector.memset(eps1, EPS_RMS)
            eps2 = const.tile([2, 1], f32, tag="eps2", name="eps2")
            nc.vector.memset(eps2, EPS_L2 * EPS_L2)
            b1_s = const.tile([128, NF], f32, tag="b1", name="b1")
            nc.sync.dma_start(out=b1_s, in_=b1_d[:, :])
            if not skip_pb:
                pb_row = const.tile([1, D], fr, tag="pbrow", name="pbrow")
                nc.sync.dma_start(out=pb_row, in_=pb_d[:, :])
            ones_f = const.tile([1, 128], f32, tag="onesf", name="onesf")
            nc.vector.memset(ones_f, 1.0)
            ones_r = const.tile([1, 128], fr, tag="onesr", name="onesr")
            nc.vector.tensor_copy(ones_r, ones_f)
            ones_b = const.tile([1, 128], bf16, tag="onesb", name="onesb")
            nc.vector.tensor_copy(ones_b, ones_f)
            if not skip_b2:
                b2_row = const.tile([1, D], bf16, tag="b2row", name="b2row")
                nc.sync.dma_start(out=b2_row, in_=b2_d[:, :])


            # h tiles double as h2 (residual accumulates in place)
            hpb = [hpbp.tile([128, D], fr, tag=f"h_{t}", name=f"h_{t}") for t in range(NT)]
            srec = [None] * NT
            for t in range(NT):
                nc.sync.dma_start(out=hpb[t], in_=h_r[t])

            hs2T_cm = tc.tile_pool(name="hs2T", bufs=1)
            hs2Tp = hs2T_cm.__enter__()
            hs2T = [hs2Tp.tile([128, S], bf16, tag=f"hs2T_{k}", name=f"hs2T_{k}") for k in range(ND)]
            g0_cm = tc.tile_pool(name="g0", bufs=1)
            g0p = g0_cm.__enter__()
            g0 = [g0p.tile([128, 512], bf16, tag=f"g0_{ft}", name=f"g0_{ft}") for ft in range(NF)]
            with (
                tc.tile_pool(name="hvp", bufs=1) as hvp,
                tc.tile_pool(name="chk", bufs=1) as chk,
                tc.tile_pool(name="ustp", bufs=1) as ustp,
            ):
                hv = [hvp.tile([128, D], bf16, tag=f"hv_{c}", name=f"hv_{c}") for c in range(NCH)]
                hs2p_tiles = [hvp.tile([128, D], bf16, tag=f"hs2_{t}", name=f"hs2_{t}") for t in range(NT)]

                with (
                    tc.tile_pool(name="hT", bufs=1) as hTp,
                    tc.tile_pool(name="prj", bufs=1) as prjp,
                    tc.tile_pool(name="qn", bufs=1) as qnp,
                ):
                    hT = [hTp.tile([128, S], bf16, tag=f"hT_{k}", name=f"hT_{k}") for k in range(ND)]
                    prj = [prjp.tile([128, D], bf16, tag=f"prj_{k}", name=f"prj_{k}") for k in range(ND)]
                    for k in range(ND):
                        nc.sync.dma_start(out=prj[k], in_=prj_r[k])

                    # ====== phase A+B: rmsnorm scales + h transposes ======
                    with tc.tile_pool(name="psT", bufs=4, space="PSUM") as psT:
                        for t in range(NT):
                            tsl = bass.ts(t, 128)
                            sq = scr1.tile([128, D], f32, tag="scr", name="sq")
                            nc.vector.tensor_tensor(out=sq, in0=hpb[t], in1=hpb[t],
                                                    op=ALU.mult)
                            ssq = scal.tile([128, 1], f32, tag="ssq", name="ssq")
                            nc.vector.tensor_reduce(ssq, sq, axis=mybir.AxisListType.X,
                                                    op=ALU.add)
                            sroot = scal.tile([128, 1], f32, tag="sroot", name="sroot")
                            nc.scalar.activation(sroot, ssq, AF.Sqrt,
                                                 bias=eps1, scale=1.0 / D)
                            sr = srp.tile([128, 1], f32, tag=f"srec_{t}", name=f"srec_{t}")
                            nc.vector.reciprocal(sr, sroot)
                            srec[t] = sr
                            for k in range(ND):
                                tp = psT.tile([128, 128], fr, tag="tp", name="tp")
                                nc.tensor.transpose(tp, hpb[t][:, bass.ts(k, 128)], ident_r)
                                nc.scalar.copy(hT[k][:, tsl], tp)

                    # ====== phase B2: qk projection + l2 scales ======
                    qnT = qnp.tile([R2, S], bf16, tag="qnT", name="qnT")
                    knT = qnp.tile([R, S], bf16, tag="knT", name="knT")
                    with (
                        tc.tile_pool(name="psB", bufs=1, space="PSUM") as psB,
                        tc.tile_pool(name="psB2", bufs=1, space="PSUM") as psB2,
                        tc.tile_pool(name="qtmp", bufs=1) as qtmp,
                    ):
                        qk_ps = psB.tile([R2, S], f32, tag="qk", name="qk")
                        for half in range(2):
                            nsl = bass.ts(half, 512)
                            for k in range(ND):
                                nc.tensor.matmul(qk_ps[:, nsl], lhsT=uv_s[:, k, :],
                                                 rhs=hT[k][:, nsl],
                                                 start=(k == 0), stop=(k == ND - 1))
                        if debug:
                            dbg_ssq2_sb = qtmp.tile([2, S], f32, tag="dbgssq", name="dbgssq")
                        qkT = qtmp.tile([R2, S], bf16, tag="qkT", name="qkT")
                        nc.vector.tensor_copy(qkT, qk_ps)
                        qk2 = qtmp.tile([R2, S], fr, tag="qk2", name="qk2")
                        nc.scalar.activation(qk2, qk_ps, AF.Square)
                        ssq2_ps = psB2.tile([2, S], f32, tag="ssq2", name="ssq2")
                        for th in range(2):
                            nsl = bass.ts(th, 512)
                            nc.tensor.matmul(ssq2_ps[:, nsl], lhsT=eo_s, rhs=qk2[:, nsl],
                                             start=True, stop=True)
                        # rsqrt(x) = exp(-0.5 ln(x + eps^2)): ACT-only, avoids
                        # the 1024-elem-per-lane DVE reciprocal
                        srow = qtmp.tile([2, S], fr, tag="srow", name="srow")
                        if debug:
                            nc.vector.tensor_copy(dbg_ssq2_sb, ssq2_ps)
                            nc.sync.dma_start(out=dbg_ssq2[:, :], in_=dbg_ssq2_sb)
                        nc.scalar.activation(srow, ssq2_ps, AF.Ln, bias=eps2)
                        nc.scalar.activation(srow, srow, AF.Exp, scale=-0.5)
                        sc_ps = psB2.tile([R2, S], f32, tag="sc32", name="sc32")
                        for th in range(2):
                            nsl = bass.ts(th, 512)
                            nc.tensor.matmul(sc_ps[:, nsl], lhsT=e2_s, rhs=srow[:, nsl],
                                             start=True, stop=True)
                        nc.vector.tensor_tensor(out=qnT, in0=qkT, in1=sc_ps, op=ALU.mult)
                        # k rows to a base-0 tile: DVE tensor_tensor needs
                        # both SBUF inputs at the same base partition
                        nc.sync.dma_start(out=knT, in_=qnT[32:32 + R, :])
                        if debug:
                            nc.sync.dma_start(out=dbg_qnT[:, :], in_=qnT)
                            nc.sync.dma_start(out=dbg_qk2[:, :], in_=qk2)
                            nc.sync.dma_start(out=dbg_srow[:, :], in_=srow)

                    # ====== phase C0: per-chunk prep (hv, M, U) + state scan ======
                    qh = [None] * NCH
                    Ms = [None] * NCH
                    stv = [None] * NCH
                    tails = [None] * NCH
                    bp_sc = [None] * NCH
                    ust = [None] * NCH
                    with (
                        tc.tile_pool(name="psHV", bufs=2, space="PSUM") as psHV,
                        tc.tile_pool(name="psA", bufs=1, space="PSUM") as psA,
                        tc.tile_pool(name="psU", bufs=2, space="PSUM") as psU,
                        tc.tile_pool(name="psK", bufs=1, space="PSUM") as psK,
                        tc.tile_pool(name="ctmp", bufs=2) as ctmp,
                    ):
                        for c in range(NCH):
                            csl = bass.ts(c, C)
                            # hv = (h^T)^T @ proj_eff, chunk c rows
                            hv_ps = psHV.tile([128, D], f32, tag="hv", name="hv")
                            for half in range(2):
                                nsl = bass.ts(half, 512)
                                for k in range(ND):
                                    nc.tensor.matmul(hv_ps[:, nsl], lhsT=hT[k][:, csl],
                                                     rhs=prj[k][:, nsl],
                                                     start=(k == 0), stop=(k == ND - 1))
                            nc.scalar.copy(hv[c], hv_ps)
                            # q/k chunk slices with decay tables
                            qf = chk.tile([R, C], bf16, tag=f"qh_{c}", name=f"qh_{c}")
                            nc.vector.tensor_tensor(out=qf, in0=qnT[:R, csl], in1=gq_s, op=ALU.mult)
                            qh[c] = qf
                            kh = ctmp.tile([R, C], bf16, tag="kh", name="kh")
                            nc.vector.tensor_tensor(out=kh, in0=knT[:, csl], in1=gk_s, op=ALU.mult)
                            kb = ctmp.tile([R, C], f32, tag="kb", name="kb")
                            nc.vector.tensor_tensor(out=kb, in0=knT[:, csl], in1=gbar_s, op=ALU.mult)
                            # intra-chunk decay matrix M = (A*tril + Bc) * s_j
                            a_ps = psA.tile([C, C], f32, tag="aps", name="aps")
                            nc.tensor.matmul(a_ps, lhsT=kh, rhs=qh[c], start=True, stop=True)
                            am = ctmp.tile([C, C], f32, tag="am", name="am")
                            nc.vector.tensor_tensor(out=am, in0=a_ps, in1=trilm, op=ALU.mult)
                            am2 = ctmp.tile([C, C], f32, tag="am2", name="am2")
                            nc.vector.tensor_tensor(out=am2, in0=am, in1=bc_s, op=ALU.add)
                            mm = chk.tile([C, C], bf16, tag=f"M_{c}", name=f"M_{c}")
                            nc.vector.tensor_scalar_mul(mm, am2, srec[c])
                            Ms[c] = mm
                            if debug:
                                nc.sync.dma_start(out=dbg_M[c], in_=mm)
                                nc.sync.dma_start(out=dbg_hv[c], in_=hv[c])
                            # kbar^T, scaled by s_j
                            kbT_ps = psK.tile([C, R], f32, tag="kbT", name="kbT")
                            nc.tensor.transpose(kbT_ps, kb, ident[:R, :R])
                            kbT = ctmp.tile([C, R], bf16, tag="kbTs", name="kbTs")
                            nc.vector.tensor_scalar_mul(kbT, kbT_ps, srec[c])
                            # state contribution U_c (v-domain)
                            uc = ustp.tile([R, D], bf16, tag=f"ust_{c}", name=f"ust_{c}")
                            ust[c] = uc
                            for half in range(2):
                                nsl = bass.ts(half, 512)
                                u_ps = psU.tile([R, 512], f32, tag="ups", name="ups")
                                nc.tensor.matmul(u_ps, lhsT=kbT, rhs=hv[c][:, nsl],
                                                 start=True, stop=True)
                                nc.vector.tensor_copy(uc[:, nsl], u_ps)
                            if c > 0:
                                # conv tail: last 3 value rows of prev chunk at
                                # partition 0 (matmul lhsT can't sit at base 125)
                                tl = chk.tile([3, D], bf16, tag=f"tail_{c}", name=f"tail_{c}")
                                nc.sync.dma_start(out=tl, in_=hv[c - 1][125:128, :])
                                tails[c] = tl
                                stl = ctmp.tile([3, 1], f32, tag="stail", name="stail")
                                nc.sync.dma_start(out=stl, in_=srec[c - 1][125:128, :])
                                bsc = chk.tile([3, C], bf16, tag=f"bpsc_{c}", name=f"bpsc_{c}")
                                nc.vector.tensor_scalar_mul(bsc, bp_s, stl)
                                bp_sc[c] = bsc
                                # decayed state scan, in place over ust:
                                # ust[c-1] += gc * state[c-1]; the result is
                                # the state entering chunk c
                                if c == 1:
                                    stv[c] = ust[0]
                                else:
                                    tmp = ctmp.tile([R, D], bf16, tag="sctmp", name="sctmp")
                                    nc.vector.tensor_scalar_mul(tmp, stv[c - 1], gc_s)
                                    nc.vector.tensor_tensor(out=ust[c - 1], in0=ust[c - 1],
                                                            in1=tmp, op=ALU.add)
                                    stv[c] = ust[c - 1]

                # ====== phase C2: chunk mixers (post-proj domain) ======
                with tc.tile_pool(name="psM", bufs=3, space="PSUM") as psM:
                    for c in range(NCH):
                        m_ps = psM.tile([128, D], f32, tag="mps", name="mps")
                        for half in range(2):
                            nsl = bass.ts(half, 512)
                            last_pb = not skip_pb
                            nc.tensor.matmul(m_ps[:, nsl], lhsT=Ms[c], rhs=hv[c][:, nsl],
                                             start=True,
                                             stop=(not last_pb and c == 0))
                            if c > 0:
                                nc.tensor.matmul(m_ps[:, nsl], lhsT=qh[c], rhs=stv[c][:, nsl],
                                                 start=False, stop=False)
                                nc.tensor.matmul(m_ps[:, nsl], lhsT=bp_sc[c], rhs=tails[c][:, nsl],
                                                 start=False, stop=not last_pb)
                            if last_pb:
                                nc.tensor.matmul(m_ps[:, nsl], lhsT=ones_r, rhs=pb_row[:, nsl],
                                                 start=False, stop=True)
                        nc.vector.tensor_tensor(out=hpb[c], in0=m_ps, in1=hpb[c], op=ALU.add)
                        # rmsnorm2 scalar chain per tile (DVE/Pool/ACT),
                        # pipelined under the remaining mixers
                        sq = scr1.tile([128, D], f32, tag="scr", name="sq")
                        nc.vector.tensor_tensor(out=sq, in0=hpb[c], in1=hpb[c],
                                                op=ALU.mult)
                        ssq = scal.tile([128, 1], f32, tag="ssq", name="ssq")
                        nc.vector.tensor_reduce(ssq, sq, axis=mybir.AxisListType.X,
                                                op=ALU.add)
                        sroot2 = scal.tile([128, 1], f32, tag="sroot", name="sroot2")
                        nc.scalar.activation(sroot2, ssq, AF.Sqrt,
                                             bias=eps1, scale=1.0 / D)
                        sr2 = scal.tile([128, 1], f32, tag="srec2", name="srec2")
                        nc.vector.reciprocal(sr2, sroot2)
                        hs2_t = hs2p_tiles[c]
                        nc.vector.tensor_scalar_mul(hs2_t, hpb[c], sr2)
                        if debug:
                            nc.sync.dma_start(
                                out=dbg_h2.rearrange("(n p) d -> n p d", p=128)[c],
                                in_=hpb[c])

                # ====== phase D2: transposes, interleaved with E1 half-0 ======
                # After tile 3, tokens 0..511 are fully transposed, so the
                # MLP up-projection for the low S-half streams dense matmuls
                # through the remaining transpose work (keeps HAM at 8/8).
                with (
                    tc.tile_pool(name="psT2", bufs=3, space="PSUM") as psT2,
                    tc.tile_pool(name="psGa", bufs=2, space="PSUM") as psGa,
                    tc.tile_pool(name="w1a", bufs=3) as w1ap,
                ):
                    def d2_tile(t):
                        tsl = bass.ts(t, 128)
                        for k in range(ND):
                            tp_ps = psT2.tile([128, 128], bf16, tag="tps", name="tps")
                            nc.tensor.transpose(tp_ps, hs2p_tiles[t][:, bass.ts(k, 128)], ident_b)
                            nc.scalar.copy(hs2T[k][:, tsl], tp_ps)

                    def e1_half0(fts):
                        for ft in fts:
                            w1_t = w1ap.tile([128, ND * 128], bf16, tag="w1t", name="w1t")
                            nc.sync.dma_start(out=w1_t, in_=w1_d[ft])
                            g_ps = psGa.tile([128, 512], f32, tag="gps", name="gps")
                            for k in range(ND):
                                nc.tensor.matmul(
                                    g_ps, lhsT=w1_t[:, bass.ts(k, 128)],
                                    rhs=hs2T[k][:, 0:512],
                                    start=(k == 0), stop=(k == ND - 1))
                            nc.scalar.activation(g0[ft], g_ps, AF.Gelu_apprx_tanh,
                                                 bias=b1_s[:, ft:ft + 1])

                    for t in range(4):
                        d2_tile(t)
                    for i, t in enumerate(range(4, NT)):
                        d2_tile(t)
                        e1_half0(range(i * 8, (i + 1) * 8))

            if True:
                # ====== phase E1b: MLP up-proj + gelu, high S-half ======
                with (
                    tc.tile_pool(name="gpool", bufs=1) as gp,
                    tc.tile_pool(name="w2s", bufs=1) as w2sp,
                ):
                    g1 = [gp.tile([128, 512], bf16, tag=f"g1_{ft}", name=f"g1_{ft}") for ft in range(NF)]
                    w2_s = [w2sp.tile([128, D], bf16, tag=f"w2_{ft}", name=f"w2_{ft}") for ft in range(NF)]
                    with (
                        tc.tile_pool(name="w1s", bufs=3) as w1sp,
                        tc.tile_pool(name="psG", bufs=2, space="PSUM") as psG,
                    ):
                        for ft in range(NF):
                            nc.sync.dma_start(out=w2_s[ft], in_=w2_r[ft])
                            w1_t = w1sp.tile([128, ND * 128], bf16, tag="w1t", name="w1t")
                            nc.sync.dma_start(out=w1_t, in_=w1_d[ft])
                            g_ps = psG.tile([128, 512], f32, tag="gps", name="gps")
                            for k in range(ND):
                                nc.tensor.matmul(
                                    g_ps, lhsT=w1_t[:, bass.ts(k, 128)],
                                    rhs=hs2T[k][:, 512:1024],
                                    start=(k == 0), stop=(k == ND - 1))
                            nc.scalar.activation(g1[ft], g_ps, AF.Gelu_apprx_tanh,
                                                 bias=b1_s[:, ft:ft + 1])

                    # ====== phase E2: MLP down-proj, PSUM-accumulated ======
                    with tc.tile_pool(name="psW", bufs=3, space="PSUM") as psW:
                        for tq in range(NT):
                            gh = g0 if tq < 4 else g1
                            tqs = bass.ts(tq % 4, 128)
                            p_w = psW.tile([128, D], f32, tag="pw", name="pw")
                            for half in range(2):
                                nsl = bass.ts(half, 512)
                                for ft in range(NF):
                                    nc.tensor.matmul(p_w[:, nsl], lhsT=gh[ft][:, tqs],
                                                     rhs=w2_s[ft][:, nsl],
                                                     start=(ft == 0),
                                                     stop=(skip_b2 and ft == NF - 1))
                                if not skip_b2:
                                    nc.tensor.matmul(p_w[:, nsl], lhsT=ones_b,
                                                     rhs=b2_row[:, nsl],
                                                     start=False, stop=True)
                            nc.vector.tensor_tensor(out=hpb[tq], in0=p_w, in1=hpb[tq],
                                                    op=ALU.add)
                            nc.sync.dma_start(out=y_r[tq], in_=hpb[tq])

            g0_cm.__exit__(None, None, None)
            hs2T_cm.__exit__(None, None, None)

    split_drain_waits(nc)
    return nc


def _make_in_maps(inputs):
    prep = _host_prep(inputs)
    h = np.ascontiguousarray(np.asarray(inputs['h'], np.float32))
    base = dict(uv_eff=prep['uv_eff'], proj_eff=prep['proj_eff'], w1p=prep['w1p'],
                w2=prep['w2'], proj_b=prep['proj_b'], b1p=prep['b1p'], b2=prep['b2'],
                Gq=prep['Gq'], Gk=prep['Gk'], Gbar=prep['Gbar'], Gc=prep['Gc'],
                Bc=prep['Bc'], Bp=prep['Bp'], eones=prep['eones'], e2=prep['e2'])
    return [dict(base, h=np.ascontiguousarray(h[b])) for b in range(B)]


def _get_nc(inputs):
    key = (not np.any(np.asarray(inputs['proj_b'])),
           not np.any(np.asarray(inputs['mlp_b2'])))
    if key not in _cache:
        _cache[key] = _build_nc(skip_pb=key[0], skip_b2=key[1])
    return _cache[key]


def kernel(**inputs):
    nc = _get_nc(inputs)
    in_maps = _make_in_maps(inputs)
    res = run_bass_kernel_spmd(nc, in_maps, core_ids=list(range(B)))
    return np.stack([res.results[b]['y'] for b in range(B)]).astype(np.float32)
